# revision 1
# baseline (speedup 1.0000x reference)
"""DBLoss (OHEM-masked BCE + masked L1 threshold loss) on 8 Trainium2 cores.

Shapes are hardcoded for the nn_DBLoss problem:
  outputs             [16, 3, 640, 640] f32
  gt_shrink_labels    [16, 640, 640]    f32
  gt_threshold_labels [16, 640, 640]    f32
Returns np.float32[4] = (loss_all, loss_shrink, loss_binary, loss_thresh).

Sharding: pure data parallel — 2 images per core, 8 cores. Each core computes
per-image partial sums (per-partition [128] vectors); the host reduces the
tiny partials and forms the masked means.

Math notes (device fast path):
 * OHEM: with neg_num == neg_total (i.e. 3*pos_num >= neg_total) the top-k
   threshold is the minimum negative score, so the selection mask is exactly
   all-ones for every valid image. The host verifies this condition per image
   (along with pos_num>0, neg_total>0) and falls back to an exact numpy
   implementation if any image needs a true top-k (cannot happen for the
   problem's uniform-random labels).
 * BCE with binarized target t and no sigmoid clipping reduces to
   softplus(x) - t*x; the host verifies |logits| < 16 so the 1e-7 clip in the
   reference is inactive.
 * threshold-loss mask (gt_t>0)|(gt_s>0): the device sums over all pixels;
   the host subtracts exact corrections for the (measure-zero) pixels where
   both labels are <= 0.
"""

import sys

import numpy as np

try:
    import concourse.bass as bass
except ImportError:  # stand-alone grading dir: fall back to known repo paths
    for _p in ("/root/.axon_site/_ro/trn_rl_repo", "/opt/trn_rl_repo"):
        if _p not in sys.path:
            sys.path.append(_p)
    import concourse.bass as bass

import concourse.tile as tile
from concourse import mybir
from concourse.bass_utils import run_bass_kernel_spmd

B, H, W = 16, 640, 640
N = H * W                    # 409600 pixels / image
P = 128                      # SBUF partitions
F = N // P                   # 3200 free elements / partition
NCORES = 8
BPC = B // NCORES            # 2 images per core
ALPHA, BETA = 1.0, 10.0
F32 = mybir.dt.float32

_CACHED_NC = None


def build_nc() -> "bass.Bass":
    """Per-core raw-bass program.

    Per image: 5 HWDGE channel loads, 7 ACT table ops (exp/ln set only),
    4 big DVE ops; per-partition partial sums in one output tile.

    Raw bass (no TileContext): this walrus build encodes at most ONE attached
    sync-wait per TPB instruction and Tile's kernel-tail drain needs ~10, so
    all cross-engine ordering uses standalone wait_ge instructions
    (EventSemaphore ops, which codegen fine) with explicit semaphores.

    Load order is tuned so ACT (the busiest engine at ~41.4 us of table ops)
    starts after the first 1.6 MB load and never stalls long, and so the
    last-arriving tensors gate the least trailing work:
      tm0 g0 s0 bn0 tm1 gt0 g1 s1 gt1 bn1

    Semaphores: one per input DMA (+16 on completion), sa = ACT op counter
    (then_inc fires on write-ack, so sa>=k also guards same-engine RAW/WAW
    on ACT outputs), sv = DVE op counter, sc = bias-constant memset done,
    dout = output DMA completion. DVE clears every semaphore at the end so
    repeated executions of the loaded NEFF start from zero.
    """
    nc = bass.Bass(dynamic_dma_scratch_size=2048, enable_partition_id=False,
                   monotonic_sem_count=0)
    outs = nc.dram_tensor("outs", [BPC, 3, N], F32, kind="ExternalInput")
    gts = nc.dram_tensor("gts", [BPC, N], F32, kind="ExternalInput")
    gtt = nc.dram_tensor("gtt", [BPC, N], F32, kind="ExternalInput")
    # columns per image b: [2b]=sum softplus(shrink), [2b+1]=sum softplus(bin)
    # then [4+3b]=sum t*shrink, [5+3b]=sum t*bin, [6+3b]=sum|sig-gt|
    part = nc.dram_tensor("part", [P, 12], F32, kind="ExternalOutput")

    ag = mybir.AluOpType.is_gt
    mul = mybir.AluOpType.mult
    sub = mybir.AluOpType.subtract
    fexp = mybir.ActivationFunctionType.Exp
    fln = mybir.ActivationFunctionType.Ln
    X = mybir.AxisListType.X
    add = mybir.AluOpType.add

    from contextlib import ExitStack
    ctx = ExitStack()
    with ctx:
        sb = lambda nm, shape: ctx.enter_context(nc.sbuf_tensor(nm, shape, F32))
        sem = lambda nm: ctx.enter_context(nc.semaphore(name=nm))
        tm = [sb("tm_0", [P, F]), sb("tm_1", [P, F])]
        s = [sb("s_0", [P, F]), sb("s_1", [P, F])]
        bn = [sb("bn_0", [P, F]), sb("bn_1", [P, F])]
        g = [sb("g_0", [P, F]), sb("g_1", [P, F])]
        gt = [sb("gt_0", [P, F]), sb("gt_1", [P, F])]
        u = [sb("u_0", [P, F]), sb("u_1", [P, F])]
        eu, tr = sb("eu", [P, F]), sb("tr", [P, F])
        po = sb("po", [P, 12])
        bias1 = sb("bias1", [P, 1])
        dtm = [sem("dtm0"), sem("dtm1")]
        ds = [sem("ds0"), sem("ds1")]
        dbn = [sem("dbn0"), sem("dbn1")]
        dbnb = sem("dbnb")
        dg = [sem("dg0"), sem("dg1")]
        dgt = [sem("dgt0"), sem("dgt1")]
        dout, sa, sv, sc = (sem(nm) for nm in ("dout", "sa", "sv", "sc"))
        all_sems = (dtm + ds + dbn + dg + dgt + [dbnb, dout, sa, sv, sc])
        block = ctx.enter_context(nc.Block(no_gpsimd_drain=True))

        pf = lambda t: t.rearrange("(p f) -> p f", p=P)

        @block.sync
        def _(sync):
            loads = [
                (tm[0], outs[0, 1], dtm[0]),
                (s[0], outs[0, 0], ds[0]),
                (g[0], gts[0], dg[0]),
                (bn[0], outs[0, 2], dbn[0]),
                (tm[1], outs[1, 1], dtm[1]),
                (gt[0], gtt[0], dgt[0]),
                (s[1], outs[1, 0], ds[1]),
                (g[1], gts[1], dg[1]),
                (gt[1], gtt[1], dgt[1]),
            ]
            for dst, src, dsem in loads:
                sync.dma_start(out=dst[:, :], in_=pf(src)).then_inc(dsem, 16)
            h = F // 2
            bn1f = pf(outs[1, 2])
            sync.dma_start(out=bn[1][:, :h], in_=bn1f[:, :h]).then_inc(dbn[1], 16)
            sync.dma_start(out=bn[1][:, h:], in_=bn1f[:, h:]).then_inc(dbnb, 16)
            sync.wait_ge(sa, 7 * BPC + 2)
            sync.wait_ge(sv, 4 * BPC + 1)
            sync.dma_start(out=part[:, :], in_=po[:, :]).then_inc(dout, 16)
            for semh in all_sems:
                if semh is not dout:
                    sync.sem_clear(semh)
            sync.wait_ge(dout, 16)
            sync.sem_clear(dout)

        @block.scalar
        def _(scalar):
            sa_n = 0

            def act(out, in_, func, wait_prev=True, **kw):
                # previous-op write-ack rides as the instruction's single
                # attached sync-wait (walrus allows exactly one)
                nonlocal sa_n
                inst = nc.scalar.activation(out=out, in_=in_, func=func,
                                            **kw).then_inc(sa, 1)
                if wait_prev and sa_n >= 1:
                    inst.wait_op(sa, sa_n, "sem-ge")
                sa_n += 1

            for b in range(BPC):
                # sigmoid(tm) = exp(-ln(1 + exp(-tm))) in place in u[b]
                scalar.wait_ge(dtm[b], 16)
                act(u[b][:, :], tm[b][:, :], fexp, wait_prev=False, scale=-1.0)
                if b == 0:
                    scalar.wait_ge(sc, 1)
                act(u[b][:, :], u[b][:, :], fln, bias=bias1[:, :])
                act(u[b][:, :], u[b][:, :], fexp, scale=-1.0)
                # BCE softplus sums: ln(1 + exp(x)), accumulated per partition
                scalar.wait_ge(ds[b], 16)
                act(eu[:, :], s[b][:, :], fexp)
                act(eu[:, :], eu[:, :], fln, bias=bias1[:, :],
                    accum_out=po[:, 2 * b : 2 * b + 1])
                if b == 0:
                    scalar.wait_ge(dbn[b], 16)
                    act(eu[:, :], bn[b][:, :], fexp)
                    act(eu[:, :], eu[:, :], fln, bias=bias1[:, :],
                        accum_out=po[:, 1:2])
                else:
                    # bn1 arrives last: process halves as they land
                    h = F // 2
                    scalar.wait_ge(dbn[b], 16)
                    act(eu[:, :h], bn[b][:, :h], fexp)
                    act(eu[:, :h], eu[:, :h], fln, bias=bias1[:, :],
                        accum_out=po[:, 3:4])
                    scalar.wait_ge(dbnb, 16)
                    act(eu[:, h:], bn[b][:, h:], fexp)
                    act(eu[:, h:], eu[:, h:], fln, bias=bias1[:, :],
                        accum_out=po[:, 4:5])
            assert sa_n == 7 * BPC + 2

        @block.vector
        def _(vector):
            nc.vector.memset(bias1[:, :], 1.0).then_inc(sc, 1)
            sv_n = 0

            def stt_sum(b, which, half=None):
                # sum (g>0.5)*x; writes (a slice of) tr
                nonlocal sv_n
                h = F // 2
                cols = {(0, "s"): 5, (0, "bn"): 6, (1, "s"): 8,
                        (1, "bn", 0): 9, (1, "bn", 1): 10}
                if half is None:
                    col = cols[(b, which)]
                    sl = slice(None)
                    dsem = ds[b] if which == "s" else dbn[b]
                else:
                    col = cols[(b, which, half)]
                    sl = slice(0, h) if half == 0 else slice(h, F)
                    dsem = dbn[b] if half == 0 else dbnb
                x = s if which == "s" else bn
                vector.wait_ge(dg[b], 16)
                vector.wait_ge(dsem, 16)
                inst = nc.vector.scalar_tensor_tensor(
                    out=tr[:, sl], in0=g[b][:, sl], scalar=0.5,
                    in1=x[b][:, sl], op0=ag, op1=mul,
                    accum_out=po[:, col : col + 1],
                ).then_inc(sv, 1)
                if sv_n >= 1:
                    inst.wait_op(sv, sv_n, "sem-ge")  # tr write-ack of prev op
                sv_n += 1

            def l1_pair(b):
                # |sigmoid - gt| summed: subtract in place into gt, abs-reduce
                nonlocal sv_n
                vector.wait_ge(sa, 7 * b + 3)   # sigmoid chain done
                vector.wait_ge(dgt[b], 16)
                nc.vector.tensor_tensor(
                    out=gt[b][:, :], in0=u[b][:, :], in1=gt[b][:, :], op=sub
                ).then_inc(sv, 1)
                sv_n += 1
                nc.vector.tensor_reduce(
                    out=po[:, 7 + 4 * b : 8 + 4 * b], in_=gt[b][:, :],
                    axis=X, op=add, apply_absolute_value=True,
                ).then_inc(sv, 1).wait_op(sv, sv_n, "sem-ge")
                sv_n += 1

            # image 0: bn arrives before gt; image 1: bn arrives last, halved
            stt_sum(0, "s")
            stt_sum(0, "bn")
            l1_pair(0)
            stt_sum(1, "s")
            l1_pair(1)
            stt_sum(1, "bn", half=0)
            stt_sum(1, "bn", half=1)
            assert sv_n == 4 * BPC + 1

    return nc


def _numpy_reference(outputs, gt_shrink_labels, gt_threshold_labels):
    """Exact fallback for inputs outside the fast-path regime."""
    OHEM_RATIO, EPS = 3, 1e-7

    def sigmoid(x):
        return 1.0 / (1.0 + np.exp(-x))

    shrink, thresh, binary = outputs[:, 0], outputs[:, 1], outputs[:, 2]
    b = outputs.shape[0]
    flat_s = shrink.reshape(b, -1)
    flat_pos = (gt_shrink_labels > 0.5).reshape(b, -1)
    n = flat_s.shape[1]
    pos_num = flat_pos.sum(axis=1)
    neg_total = n - pos_num
    neg_num = np.minimum(pos_num * OHEM_RATIO, neg_total)
    neg_scores = np.where(flat_pos, -np.inf, flat_s)
    sorted_desc = -np.sort(-neg_scores, axis=1)
    idx = np.clip(neg_num - 1, 0, n - 1).astype(np.int64)
    thr = np.take_along_axis(sorted_desc, idx[:, None], axis=1)
    mask = (flat_s >= thr) | flat_pos
    valid = (pos_num > 0) & (neg_num > 0)
    mask = (mask & valid[:, None]).reshape(shrink.shape).astype(np.float32)

    def masked_bce(logits, target, m):
        p = np.clip(sigmoid(logits), EPS, 1.0 - EPS)
        t = (target > 0.5).astype(np.float32)
        per_px = -(t * np.log(p) + (1.0 - t) * np.log(1.0 - p))
        denom = m.sum()
        return float(per_px.flatten() @ m.flatten() / max(denom, 1.0)) if denom > 0 else 0.0

    loss_shrink = masked_bce(shrink, gt_shrink_labels, mask)
    loss_binary = masked_bce(binary, gt_shrink_labels, mask)
    m2 = ((gt_threshold_labels > 0) | (gt_shrink_labels > 0)).astype(np.float32)
    denom2 = m2.sum()
    l1 = np.abs(sigmoid(thresh) - gt_threshold_labels).flatten() @ m2.flatten()
    loss_thresh = float(l1 / max(denom2, 1.0)) if denom2 > 0 else 0.0
    loss_all = loss_shrink + ALPHA * loss_binary + BETA * loss_thresh
    return np.array([loss_all, loss_shrink, loss_binary, loss_thresh], np.float32)


def kernel(outputs, gt_shrink_labels, gt_threshold_labels, _trace=False):
    global _CACHED_NC
    outputs = np.ascontiguousarray(np.asarray(outputs, dtype=np.float32))
    gts = np.ascontiguousarray(np.asarray(gt_shrink_labels, dtype=np.float32))
    gtt = np.ascontiguousarray(np.asarray(gt_threshold_labels, dtype=np.float32))

    # ---- host-side regime checks (exactness guards for the fast path) ----
    pos_num = (gts > 0.5).reshape(B, -1).sum(axis=1)
    neg_total = N - pos_num
    neg_num = np.minimum(3 * pos_num, neg_total)
    valid = (pos_num > 0) & (neg_num > 0)
    needs_topk = valid & (3 * pos_num < neg_total)
    clip_active = max(
        float(np.abs(outputs[:, 0]).max()), float(np.abs(outputs[:, 2]).max())
    ) >= 16.0
    if needs_topk.any() or clip_active:
        return _numpy_reference(outputs, gts, gtt)

    if _CACHED_NC is None:
        _CACHED_NC = build_nc()
    nc = _CACHED_NC

    in_maps = []
    for c in range(NCORES):
        sl = slice(c * BPC, (c + 1) * BPC)
        in_maps.append({
            "outs": outputs[sl].reshape(BPC, 3, N),
            "gts": gts[sl].reshape(BPC, N),
            "gtt": gtt[sl].reshape(BPC, N),
        })
    res = run_bass_kernel_spmd(
        nc, in_maps, core_ids=list(range(NCORES)), trace=_trace
    )

    # ---- host combine: per-image sums from per-partition partials ----
    sp_s = np.empty(B); sp_b = np.empty(B); ts = np.empty(B); tb = np.empty(B)
    l1 = np.empty(B)
    for c in range(NCORES):
        po = res.results[c]["part"].astype(np.float64).sum(axis=0)
        i0, i1 = c * BPC, c * BPC + 1
        sp_s[i0], sp_b[i0] = po[0], po[1]
        sp_s[i1], sp_b[i1] = po[2], po[3] + po[4]
        ts[i0], tb[i0], l1[i0] = po[5], po[6], po[7]
        ts[i1], tb[i1], l1[i1] = po[8], po[9] + po[10], po[11]

    cnt = float(N * valid.sum())
    num_s = float(((sp_s - ts) * valid).sum())
    num_b = float(((sp_b - tb) * valid).sum())
    loss_shrink = num_s / max(cnt, 1.0) if cnt > 0 else 0.0
    loss_binary = num_b / max(cnt, 1.0) if cnt > 0 else 0.0

    # threshold-loss mask corrections for pixels where both labels <= 0
    zz = (gtt <= 0) & (gts <= 0)
    cnt2 = float(B * N - zz.sum())
    l1_tot = float(l1.sum())
    if zz.any():
        tmz = outputs[:, 1][zz]
        l1_tot -= float(np.abs(1.0 / (1.0 + np.exp(-tmz)) - gtt[zz]).sum())
    loss_thresh = l1_tot / max(cnt2, 1.0) if cnt2 > 0 else 0.0

    loss_all = loss_shrink + ALPHA * loss_binary + BETA * loss_thresh
    out = np.array([loss_all, loss_shrink, loss_binary, loss_thresh], np.float32)
    if _trace:
        return out, res
    return out



# revision 26
# speedup vs baseline: 1.0714x; 1.0714x over previous
"""DBLoss (OHEM-masked BCE + masked L1 threshold loss) on 8 Trainium2 cores.

Shapes are hardcoded for the nn_DBLoss problem:
  outputs             [16, 3, 640, 640] f32
  gt_shrink_labels    [16, 640, 640]    f32
  gt_threshold_labels [16, 640, 640]    f32
Returns np.float32[4] = (loss_all, loss_shrink, loss_binary, loss_thresh).

Sharding: pure data parallel — 2 images per core, 8 cores. Each core computes
per-partition partial sums in one [128, 16] tile; the host reduces the tiny
partials and forms the masked means.

Math notes (device fast path):
 * OHEM: with neg_num == neg_total (3*pos_num >= neg_total) the selection
   mask is all-ones for every valid image. The host verifies this per image
   and falls back to exact numpy otherwise.
 * BCE with binarized target t and inactive clipping is softplus(x) - t*x;
   softplus = ln(exp(x) + 1) via the ACT natural_log_exp table set (bias=1
   rides the activation's free input affine), accumulated per partition.
 * sigmoid(tm) uses the ACT sigmoid table (in place, halves).
 * the L1 term uses sum|u-g| = 2*sum(max(u,g)) - sum(u) - sum(g):
   sum(max) is one DVE scalar_tensor_tensor(op1=max) with accumulate,
   sum(u) rides the sigmoid activations' accumulators for free, and
   sum(g) is a plain input sum the host computes — no elementwise
   subtract or abs on device at all.
 * the (g>0.5)*x masked sums are scalar_tensor_tensor ops with accumulate,
   split between DVE (shrink map) and GpSimd (binary map) so no engine's
   queue dominates the tail.
 * threshold-loss mask (gt_t>0)|(gt_s>0): the device sums over all pixels;
   the host subtracts exact corrections for the (measure-zero) pixels where
   both labels are <= 0.

Engine budget per core: ACT ~34us (2 sigmoid + 4x exp/ln softplus + 2 table
loads), DVE ~14us, GpSimd ~10us, DMA stream 16.4MB at ~420 GB/s ~= 39us.
Transfer order tm0 tm1 s0 g0 bn0 s1 bn1 g1 (halves; gtt0/gtt1 ride SWDGE
mid-stream) so each engine's inputs land just in time and only half-tensor
ops trail the last byte.
"""

import sys

import numpy as np

try:
    import concourse.bass as bass
except ImportError:  # stand-alone grading dir: fall back to known repo paths
    for _p in ("/root/.axon_site/_ro/trn_rl_repo", "/opt/trn_rl_repo"):
        if _p not in sys.path:
            sys.path.append(_p)
    import concourse.bass as bass

from concourse import mybir
from concourse.bass_utils import run_bass_kernel_spmd

B, H, W = 16, 640, 640
N = H * W                    # 409600 pixels / image
P = 128                      # SBUF partitions
F = N // P                   # 3200 free elements / partition
NCORES = 8
BPC = B // NCORES            # 2 images per core
ALPHA, BETA = 1.0, 10.0
F32 = mybir.dt.float32
NCOL = 32                    # partial-sum columns in the output tile

_CACHED_NC = None


def build_nc() -> "bass.Bass":
    """Per-core raw-bass program. See module docstring for the schedule.

    Raw bass (no TileContext): this walrus build encodes at most ONE attached
    sync-wait per TPB instruction, so cross-engine ordering uses standalone
    wait_ge instructions with explicit semaphores. Input DMAs all ride the
    sync-engine HWDGE ring in issue order, so a full (+16) wait on transfer
    k's semaphore also implies every earlier transfer completed; each
    consumer waits only on its latest-slot input.

    Output column map (per-partition partial sums):
      0: sum max(u0,gtt0) full   25..28: sum max(u1,gtt1) quarters
      1: sum softplus(s0)        2..5: sum softplus(bn0) quarters
      6: sum softplus(s1)        7/8: sum softplus(bn1) halves
      9..12: sum u halves (sigmoid accums)
      13..16: sum t0*s0 quarters 17..20: sum t0*bn0 quarters
      21/22: sum t1*s1 halves    23/24: sum t1*bn1 halves
    """
    nc = bass.Bass(dynamic_dma_scratch_size=2048, enable_partition_id=False,
                   monotonic_sem_count=0)
    outs = nc.dram_tensor("outs", [BPC, 3, N], F32, kind="ExternalInput")
    gts = nc.dram_tensor("gts", [BPC, N], F32, kind="ExternalInput")
    gtt = nc.dram_tensor("gtt", [BPC, N], F32, kind="ExternalInput")
    part = nc.dram_tensor("part", [P, NCOL], F32, kind="ExternalOutput")

    ag = mybir.AluOpType.is_gt
    mul = mybir.AluOpType.mult
    mx = mybir.AluOpType.max
    add = mybir.AluOpType.add
    X = mybir.AxisListType.X
    fsig = mybir.ActivationFunctionType.Sigmoid
    fexp = mybir.ActivationFunctionType.Exp
    fln = mybir.ActivationFunctionType.Ln

    from contextlib import ExitStack
    ctx = ExitStack()
    with ctx:
        sb = lambda nm, shape: ctx.enter_context(nc.sbuf_tensor(nm, shape, F32))
        sem = lambda nm: ctx.enter_context(nc.semaphore(name=nm))
        tm = [sb("tm_0", [P, F]), sb("tm_1", [P, F])]
        s = [sb("s_0", [P, F]), sb("s_1", [P, F])]
        bn = [sb("bn_0", [P, F]), sb("bn_1", [P, F])]
        g = [sb("g_0", [P, F]), sb("g_1", [P, F])]
        gt = [sb("gt_0", [P, F]), sb("gt_1", [P, F])]
        tra = sb("tra", [P, F])   # ACT exp/ln scratch
        trv = sb("trv", [P, F])   # DVE stt scratch
        po = sb("po", [P, NCOL])
        bias1 = sb("bias1", [P, 1])

        # one semaphore per HWDGE input transfer, in ring order
        slot_names = ["tm0a", "tm0b", "tm1a", "tm1b", "s0a", "s0b",
                      "gt0a", "gt0b",
                      "bn0q1", "g0q1", "bn0q2", "g0q2",
                      "bn0q3", "g0q3", "bn0q4", "g0q4",
                      "s1a", "s1b", "g1a", "g1b", "bn1a", "bn1b",
                      "gt1q1", "gt1q2", "gt1q3", "gt1q4"]
        dsem = {nm: sem("d_" + nm) for nm in slot_names}
        dout, sa, sv, sc = (sem(nm) for nm in ("dout", "sa", "sv", "sc"))
        all_sems = list(dsem.values()) + [dout, sa, sv, sc]
        block = ctx.enter_context(nc.Block(no_gpsimd_drain=True))

        pf = lambda t: t.rearrange("(p f) -> p f", p=P)
        h, q = F // 2, F // 4
        lo, hi = slice(0, h), slice(h, F)
        qs = [slice(i * q, (i + 1) * q) for i in range(4)]

        @block.sync
        def _(sync):
            srcs = {
                "tm0": pf(outs[0, 1]), "tm1": pf(outs[1, 1]),
                "s0": pf(outs[0, 0]), "s1": pf(outs[1, 0]),
                "bn0": pf(outs[0, 2]), "bn1": pf(outs[1, 2]),
                "g0": pf(gts[0]), "g1": pf(gts[1]),
                "gt0": pf(gtt[0]), "gt1": pf(gtt[1]),
            }
            tiles = {"tm0": tm[0], "tm1": tm[1], "s0": s[0], "s1": s[1],
                     "bn0": bn[0], "bn1": bn[1], "g0": g[0], "g1": g[1],
                     "gt0": gt[0], "gt1": gt[1]}
            for nm in slot_names:
                if nm.endswith(("a", "b")):
                    base, sl = nm[:-1], (lo if nm[-1] == "a" else hi)
                else:
                    base, sl = nm[:-2], qs[int(nm[-1]) - 1]
                sync.dma_start(out=tiles[base][:, sl],
                               in_=srcs[base][:, sl]).then_inc(dsem[nm], 16)
            sync.wait_ge(sa, 20)
            sync.wait_ge(sv, 17)
            sync.dma_start(out=part[:, :], in_=po[:, :]).then_inc(dout, 16)
            for semh in all_sems:
                if semh is not dout:
                    sync.sem_clear(semh)
            sync.wait_ge(dout, 16)
            sync.sem_clear(dout)

        @block.scalar
        def _(scalar):
            sa_n = 0

            def act(out, in_, func, col=None, wait=None, **kw):
                nonlocal sa_n
                if wait is not None:
                    scalar.wait_ge(wait, 16)
                if col is not None:
                    kw["accum_out"] = po[:, col : col + 1]
                inst = nc.scalar.activation(out=out, in_=in_, func=func,
                                            **kw).then_inc(sa, 1)
                if sa_n >= 1:
                    inst.wait_op(sa, sa_n, "sem-ge")
                sa_n += 1

            act(tm[0][:, lo], tm[0][:, lo], fsig, col=9, wait=dsem["tm0a"])
            act(tm[0][:, hi], tm[0][:, hi], fsig, col=10, wait=dsem["tm0b"])
            act(tm[1][:, lo], tm[1][:, lo], fsig, col=11, wait=dsem["tm1a"])
            act(tm[1][:, hi], tm[1][:, hi], fsig, col=12, wait=dsem["tm1b"])
            # softplus sums: ln(exp(x)*1 + 1) accumulated per partition
            scalar.wait_ge(sc, 1)
            act(tra[:, :], s[0][:, :], fexp, wait=dsem["s0b"])
            act(tra[:, :], tra[:, :], fln, bias=bias1[:, :], col=1)
            for i in range(4):  # bn0 quarters chase the interleaved stream
                act(tra[:, qs[i]], bn[0][:, qs[i]], fexp,
                    wait=dsem[f"bn0q{i + 1}"])
                act(tra[:, qs[i]], tra[:, qs[i]], fln, bias=bias1[:, :],
                    col=2 + i)
            act(tra[:, :], s[1][:, :], fexp, wait=dsem["s1b"])
            act(tra[:, :], tra[:, :], fln, bias=bias1[:, :], col=6)
            act(tra[:, lo], bn[1][:, lo], fexp, wait=dsem["bn1a"])
            act(tra[:, lo], tra[:, lo], fln, bias=bias1[:, :], col=7)
            act(tra[:, hi], bn[1][:, hi], fexp, wait=dsem["bn1b"])
            act(tra[:, hi], tra[:, hi], fln, bias=bias1[:, :], col=8)
            assert sa_n == 20

        @block.vector
        def _(vector):
            nc.vector.memset(bias1[:, :], 1.0).then_inc(sc, 1)
            sv_n = 0

            def chain(inst):
                nonlocal sv_n
                inst.then_inc(sv, 1)
                if sv_n >= 1:
                    inst.wait_op(sv, sv_n, "sem-ge")
                sv_n += 1

            def stt(xt, gt_, sl, col, wait):
                vector.wait_ge(wait, 16)
                chain(nc.vector.scalar_tensor_tensor(
                    out=trv[:, sl], in0=gt_[:, sl], scalar=0.5,
                    in1=xt[:, sl], op0=ag, op1=mul,
                    accum_out=po[:, col : col + 1],
                ))

            def stt_max(ut, gtt_t, sl, col, sa_min, wait):
                # sum max(sigmoid(tm), gtt): (u * 1.0) max gtt, accumulated
                vector.wait_ge(sa, sa_min)
                vector.wait_ge(wait, 16)
                chain(nc.vector.scalar_tensor_tensor(
                    out=trv[:, sl], in0=ut[:, sl], scalar=1.0,
                    in1=gtt_t[:, sl], op0=mul, op1=mx,
                    accum_out=po[:, col : col + 1],
                ))

            stt_max(tm[0], gt[0], slice(None), 0, 2, dsem["gt0b"])
            for i in range(4):  # s0/bn0 masked sums chase the g0 quarters
                stt(s[0], g[0], qs[i], 13 + i, dsem[f"g0q{i + 1}"])
                stt(bn[0], g[0], qs[i], 17 + i, dsem[f"g0q{i + 1}"])
            stt(s[1], g[1], lo, 21, dsem["g1a"])
            stt(s[1], g[1], hi, 22, dsem["g1b"])
            stt(bn[1], g[1], lo, 23, dsem["bn1a"])
            stt(bn[1], g[1], hi, 24, dsem["bn1b"])
            for i in range(4):  # gtt1 quarters are the stream tail
                stt_max(tm[1], gt[1], qs[i], 25 + i, 4, dsem[f"gt1q{i + 1}"])
            assert sv_n == 17

    return nc


def _numpy_reference(outputs, gt_shrink_labels, gt_threshold_labels):
    """Exact fallback for inputs outside the fast-path regime."""
    OHEM_RATIO, EPS = 3, 1e-7

    def sigmoid(x):
        return 1.0 / (1.0 + np.exp(-x))

    shrink, thresh, binary = outputs[:, 0], outputs[:, 1], outputs[:, 2]
    b = outputs.shape[0]
    flat_s = shrink.reshape(b, -1)
    flat_pos = (gt_shrink_labels > 0.5).reshape(b, -1)
    n = flat_s.shape[1]
    pos_num = flat_pos.sum(axis=1)
    neg_total = n - pos_num
    neg_num = np.minimum(pos_num * OHEM_RATIO, neg_total)
    neg_scores = np.where(flat_pos, -np.inf, flat_s)
    sorted_desc = -np.sort(-neg_scores, axis=1)
    idx = np.clip(neg_num - 1, 0, n - 1).astype(np.int64)
    thr = np.take_along_axis(sorted_desc, idx[:, None], axis=1)
    mask = (flat_s >= thr) | flat_pos
    valid = (pos_num > 0) & (neg_num > 0)
    mask = (mask & valid[:, None]).reshape(shrink.shape).astype(np.float32)

    def masked_bce(logits, target, m):
        p = np.clip(sigmoid(logits), EPS, 1.0 - EPS)
        t = (target > 0.5).astype(np.float32)
        per_px = -(t * np.log(p) + (1.0 - t) * np.log(1.0 - p))
        denom = m.sum()
        return float(per_px.flatten() @ m.flatten() / max(denom, 1.0)) if denom > 0 else 0.0

    loss_shrink = masked_bce(shrink, gt_shrink_labels, mask)
    loss_binary = masked_bce(binary, gt_shrink_labels, mask)
    m2 = ((gt_threshold_labels > 0) | (gt_shrink_labels > 0)).astype(np.float32)
    denom2 = m2.sum()
    l1 = np.abs(sigmoid(thresh) - gt_threshold_labels).flatten() @ m2.flatten()
    loss_thresh = float(l1 / max(denom2, 1.0)) if denom2 > 0 else 0.0
    loss_all = loss_shrink + ALPHA * loss_binary + BETA * loss_thresh
    return np.array([loss_all, loss_shrink, loss_binary, loss_thresh], np.float32)


def kernel(outputs, gt_shrink_labels, gt_threshold_labels, _trace=False):
    global _CACHED_NC
    outputs = np.ascontiguousarray(np.asarray(outputs, dtype=np.float32))
    gts = np.ascontiguousarray(np.asarray(gt_shrink_labels, dtype=np.float32))
    gtt = np.ascontiguousarray(np.asarray(gt_threshold_labels, dtype=np.float32))

    # ---- host-side regime checks (exactness guards for the fast path) ----
    pos_num = (gts > 0.5).reshape(B, -1).sum(axis=1)
    neg_total = N - pos_num
    neg_num = np.minimum(3 * pos_num, neg_total)
    valid = (pos_num > 0) & (neg_num > 0)
    needs_topk = valid & (3 * pos_num < neg_total)
    clip_active = max(
        float(np.abs(outputs[:, 0]).max()), float(np.abs(outputs[:, 2]).max())
    ) >= 16.0
    if needs_topk.any() or clip_active or not valid.all():
        return _numpy_reference(outputs, gts, gtt)

    if _CACHED_NC is None:
        _CACHED_NC = build_nc()
    nc = _CACHED_NC

    in_maps = []
    for c in range(NCORES):
        sl = slice(c * BPC, (c + 1) * BPC)
        in_maps.append({
            "outs": outputs[sl].reshape(BPC, 3, N),
            "gts": gts[sl].reshape(BPC, N),
            "gtt": gtt[sl].reshape(BPC, N),
        })
    res = run_bass_kernel_spmd(
        nc, in_maps, core_ids=list(range(NCORES)), trace=_trace
    )

    # ---- host combine: global sums from per-partition partials ----
    # sum(gtt) is a plain input reduction; the host computes it directly
    sum_g_all = float(gtt.astype(np.float64).sum())
    sp_s = sp_b = ts = tb = 0.0
    l1 = -sum_g_all
    for c in range(NCORES):
        po = res.results[c]["part"].astype(np.float64).sum(axis=0)
        sum_max = po[0] + po[25] + po[26] + po[27] + po[28]
        sum_u = po[9] + po[10] + po[11] + po[12]
        l1 += 2.0 * sum_max - sum_u
        sp_s += po[1] + po[6]
        sp_b += po[2] + po[3] + po[4] + po[5] + po[7] + po[8]
        ts += po[13] + po[14] + po[15] + po[16] + po[21] + po[22]
        tb += po[17] + po[18] + po[19] + po[20] + po[23] + po[24]

    cnt = float(B * N)
    loss_shrink = (sp_s - ts) / cnt
    loss_binary = (sp_b - tb) / cnt

    # threshold-loss mask corrections for pixels where both labels <= 0
    zz = (gtt <= 0) & (gts <= 0)
    cnt2 = float(B * N - zz.sum())
    if zz.any():
        tmz = outputs[:, 1][zz]
        l1 -= float(np.abs(1.0 / (1.0 + np.exp(-tmz)) - gtt[zz]).sum())
    loss_thresh = l1 / max(cnt2, 1.0) if cnt2 > 0 else 0.0

    loss_all = loss_shrink + ALPHA * loss_binary + BETA * loss_thresh
    out = np.array([loss_all, loss_shrink, loss_binary, loss_thresh], np.float32)
    if _trace:
        return out, res
    return out


# revision 34
# speedup vs baseline: 1.2022x; 1.1221x over previous
"""DBLoss (OHEM-masked BCE + masked L1 threshold loss) on 8 Trainium2 cores.

Shapes are hardcoded for the nn_DBLoss problem:
  outputs             [16, 3, 640, 640] f32
  gt_shrink_labels    [16, 640, 640]    f32
  gt_threshold_labels [16, 640, 640]    f32
Returns np.float32[4] = (loss_all, loss_shrink, loss_binary, loss_thresh).

Sharding: pure data parallel — 2 images per core, 8 cores. Each core computes
per-partition partial sums in one [128, 16] tile; the host reduces the tiny
partials and forms the masked means.

Math notes (device fast path):
 * OHEM: with neg_num == neg_total (3*pos_num >= neg_total) the selection
   mask is all-ones for every valid image. The host verifies this per image
   and falls back to exact numpy otherwise.
 * BCE with binarized target t and inactive clipping is softplus(x) - t*x;
   softplus = ln(exp(x) + 1) via the ACT natural_log_exp table set (bias=1
   rides the activation's free input affine), accumulated per partition.
 * sigmoid(tm) uses the ACT sigmoid table (in place, halves).
 * the L1 term uses sum|u-g| = 2*sum(max(u,g)) - sum(u) - sum(g):
   sum(max) is one DVE scalar_tensor_tensor(op1=max) with accumulate,
   sum(u) rides the sigmoid activations' accumulators for free, and
   sum(g) is a plain input sum the host computes — no elementwise
   subtract or abs on device at all.
 * the (g>0.5)*x masked sums are scalar_tensor_tensor ops with accumulate,
   split between DVE (shrink map) and GpSimd (binary map) so no engine's
   queue dominates the tail.
 * threshold-loss mask (gt_t>0)|(gt_s>0): the device sums over all pixels;
   the host subtracts exact corrections for the (measure-zero) pixels where
   both labels are <= 0.

Engine budget per core: ACT ~34us (2 sigmoid + 4x exp/ln softplus + 2 table
loads), DVE ~14us, GpSimd ~10us, DMA stream 16.4MB at ~420 GB/s ~= 39us.
Transfer order tm0 tm1 s0 g0 bn0 s1 bn1 g1 (halves; gtt0/gtt1 ride SWDGE
mid-stream) so each engine's inputs land just in time and only half-tensor
ops trail the last byte.
"""

import sys

import numpy as np

try:
    import concourse.bass as bass
except ImportError:  # stand-alone grading dir: fall back to known repo paths
    for _p in ("/root/.axon_site/_ro/trn_rl_repo", "/opt/trn_rl_repo"):
        if _p not in sys.path:
            sys.path.append(_p)
    import concourse.bass as bass

from concourse import mybir
from concourse.bass_utils import run_bass_kernel_spmd

B, H, W = 16, 640, 640
N = H * W                    # 409600 pixels / image
P = 128                      # SBUF partitions
F = N // P                   # 3200 free elements / partition
NCORES = 8
BPC = B // NCORES            # 2 images per core
ALPHA, BETA = 1.0, 10.0
F32 = mybir.dt.float32
NCOL = 32                    # partial-sum columns in the output tile

_CACHED_NC = None


def build_nc() -> "bass.Bass":
    """Per-core raw-bass program. See module docstring for the schedule.

    Raw bass (no TileContext): this walrus build encodes at most ONE attached
    sync-wait per TPB instruction, so cross-engine ordering uses standalone
    wait_ge instructions with explicit semaphores. Input DMAs all ride the
    sync-engine HWDGE ring in issue order, so a full (+16) wait on transfer
    k's semaphore also implies every earlier transfer completed; each
    consumer waits only on its latest-slot input.

    Output column map (per-partition partial sums):
      0: sum max(u0,gtt0) full   25..28: sum max(u1,gtt1) quarters
      1: sum softplus(s0)        2/3: sum softplus(bn0) halves
      6: sum softplus(s1)        7/8: sum softplus(bn1) halves
      9/10: sum u0 halves        11: sum u1 (sigmoid accums)
      13..16: sum t0*s0 quarters 17..20: sum t0*bn0 quarters
      21/22: sum t1*s1 halves    23/24: sum t1*bn1 halves
    """
    nc = bass.Bass(dynamic_dma_scratch_size=2048, enable_partition_id=False,
                   monotonic_sem_count=0)
    outs = nc.dram_tensor("outs", [BPC, 3, N], F32, kind="ExternalInput")
    gts = nc.dram_tensor("gts", [BPC, N], F32, kind="ExternalInput")
    gtt = nc.dram_tensor("gtt", [BPC, N], F32, kind="ExternalInput")
    part = nc.dram_tensor("part", [P, NCOL], F32, kind="ExternalOutput")

    ag = mybir.AluOpType.is_gt
    mul = mybir.AluOpType.mult
    mx = mybir.AluOpType.max
    add = mybir.AluOpType.add
    X = mybir.AxisListType.X
    fsig = mybir.ActivationFunctionType.Sigmoid
    fexp = mybir.ActivationFunctionType.Exp
    fln = mybir.ActivationFunctionType.Ln

    from contextlib import ExitStack
    ctx = ExitStack()
    with ctx:
        sb = lambda nm, shape: ctx.enter_context(nc.sbuf_tensor(nm, shape, F32))
        sem = lambda nm: ctx.enter_context(nc.semaphore(name=nm))
        tm = [sb("tm_0", [P, F]), sb("tm_1", [P, F])]
        s = [sb("s_0", [P, F]), sb("s_1", [P, F])]
        bn = [sb("bn_0", [P, F]), sb("bn_1", [P, F])]
        g = [sb("g_0", [P, F]), sb("g_1", [P, F])]
        gt = [sb("gt_0", [P, F]), sb("gt_1", [P, F])]
        tra = sb("tra", [P, F])   # ACT exp/ln scratch
        trv = sb("trv", [P, F])   # DVE stt scratch
        dmy = sb("dmy", [P, 1])   # table-load dummy scratch
        po = sb("po", [P, NCOL])
        bias1 = sb("bias1", [P, 1])

        # one semaphore per HWDGE input transfer, in ring order
        slot_names = ["tm0a", "tm0b", "tm1a", "tm1b", "s0a", "s0b",
                      "gt0a", "gt0b",
                      "bn0q1", "g0q1", "bn0q2", "g0q2",
                      "bn0q3", "g0q3", "bn0q4", "g0q4",
                      "s1a", "s1b", "g1a", "g1b", "bn1a", "bn1b",
                      "gt1q1", "gt1q2", "gt1q3", "gt1q4"]
        dsem = {nm: sem("d_" + nm) for nm in slot_names}
        dout, sa, sv, sc = (sem(nm) for nm in ("dout", "sa", "sv", "sc"))
        all_sems = list(dsem.values()) + [dout, sa, sv, sc]
        block = ctx.enter_context(nc.Block(no_gpsimd_drain=True))

        pf = lambda t: t.rearrange("(p f) -> p f", p=P)
        h, q = F // 2, F // 4
        lo, hi = slice(0, h), slice(h, F)
        qs = [slice(i * q, (i + 1) * q) for i in range(4)]

        @block.sync
        def _(sync):
            srcs = {
                "tm0": pf(outs[0, 1]), "tm1": pf(outs[1, 1]),
                "s0": pf(outs[0, 0]), "s1": pf(outs[1, 0]),
                "bn0": pf(outs[0, 2]), "bn1": pf(outs[1, 2]),
                "g0": pf(gts[0]), "g1": pf(gts[1]),
                "gt0": pf(gtt[0]), "gt1": pf(gtt[1]),
            }
            tiles = {"tm0": tm[0], "tm1": tm[1], "s0": s[0], "s1": s[1],
                     "bn0": bn[0], "bn1": bn[1], "g0": g[0], "g1": g[1],
                     "gt0": gt[0], "gt1": gt[1]}
            for nm in slot_names:
                if nm.endswith(("a", "b")):
                    base, sl = nm[:-1], (lo if nm[-1] == "a" else hi)
                else:
                    base, sl = nm[:-2], qs[int(nm[-1]) - 1]
                sync.dma_start(out=tiles[base][:, sl],
                               in_=srcs[base][:, sl]).then_inc(dsem[nm], 16)
            sync.wait_ge(sa, 15)
            sync.wait_ge(sv, 17)
            sync.dma_start(out=part[:, :], in_=po[:, :]).then_inc(dout, 16)
            for semh in all_sems:
                if semh is not dout:
                    sync.sem_clear(semh)
            sync.wait_ge(dout, 16)
            sync.sem_clear(dout)

        @block.scalar
        def _(scalar):
            sa_n = 0

            def act(out, in_, func, col=None, wait=None, inc=True, **kw):
                nonlocal sa_n
                if wait is not None:
                    scalar.wait_ge(wait, 16)
                if col is not None:
                    kw["accum_out"] = po[:, col : col + 1]
                inst = nc.scalar.activation(out=out, in_=in_, func=func, **kw)
                if inc:
                    inst.then_inc(sa, 1)
                    if sa_n >= 1:
                        inst.wait_op(sa, sa_n, "sem-ge")
                    sa_n += 1

            # no-wait dummy pulls the sigmoid table load into idle time
            act(dmy[:, :], dmy[:, :], fsig, inc=False)
            act(tm[0][:, lo], tm[0][:, lo], fsig, col=9, wait=dsem["tm0a"])
            act(tm[0][:, hi], tm[0][:, hi], fsig, col=10, wait=dsem["tm0b"])
            act(tm[1][:, :], tm[1][:, :], fsig, col=11, wait=dsem["tm1b"])
            # no-wait dummy pulls the exp/ln table switch right after sigmoid
            act(dmy[:, :], dmy[:, :], fexp, inc=False)
            # softplus sums: ln(exp(x)*1 + 1) accumulated per partition
            scalar.wait_ge(sc, 1)
            act(tra[:, :], s[0][:, :], fexp, wait=dsem["s0b"])
            act(tra[:, :], tra[:, :], fln, bias=bias1[:, :], col=1)
            # bn0 streams as quarters for DVE pacing; ACT eats it in halves
            act(tra[:, lo], bn[0][:, lo], fexp, wait=dsem["bn0q2"])
            act(tra[:, lo], tra[:, lo], fln, bias=bias1[:, :], col=2)
            act(tra[:, hi], bn[0][:, hi], fexp, wait=dsem["bn0q4"])
            act(tra[:, hi], tra[:, hi], fln, bias=bias1[:, :], col=3)
            act(tra[:, :], s[1][:, :], fexp, wait=dsem["s1b"])
            act(tra[:, :], tra[:, :], fln, bias=bias1[:, :], col=6)
            act(tra[:, lo], bn[1][:, lo], fexp, wait=dsem["bn1a"])
            act(tra[:, lo], tra[:, lo], fln, bias=bias1[:, :], col=7)
            act(tra[:, hi], bn[1][:, hi], fexp, wait=dsem["bn1b"])
            act(tra[:, hi], tra[:, hi], fln, bias=bias1[:, :], col=8)
            assert sa_n == 15

        @block.vector
        def _(vector):
            nc.vector.memset(bias1[:, :], 1.0).then_inc(sc, 1)
            sv_n = 0

            def chain(inst):
                nonlocal sv_n
                inst.then_inc(sv, 1)
                if sv_n >= 1:
                    inst.wait_op(sv, sv_n, "sem-ge")
                sv_n += 1

            def stt(xt, gt_, sl, col, wait):
                vector.wait_ge(wait, 16)
                chain(nc.vector.scalar_tensor_tensor(
                    out=trv[:, sl], in0=gt_[:, sl], scalar=0.5,
                    in1=xt[:, sl], op0=ag, op1=mul,
                    accum_out=po[:, col : col + 1],
                ))

            def stt_max(ut, gtt_t, sl, col, sa_min, wait):
                # sum max(sigmoid(tm), gtt): (u * 1.0) max gtt, accumulated
                vector.wait_ge(sa, sa_min)
                vector.wait_ge(wait, 16)
                chain(nc.vector.scalar_tensor_tensor(
                    out=trv[:, sl], in0=ut[:, sl], scalar=1.0,
                    in1=gtt_t[:, sl], op0=mul, op1=mx,
                    accum_out=po[:, col : col + 1],
                ))

            stt_max(tm[0], gt[0], slice(None), 0, 2, dsem["gt0b"])
            for i in range(4):  # s0/bn0 masked sums chase the g0 quarters
                stt(s[0], g[0], qs[i], 13 + i, dsem[f"g0q{i + 1}"])
                stt(bn[0], g[0], qs[i], 17 + i, dsem[f"g0q{i + 1}"])
            stt(s[1], g[1], lo, 21, dsem["g1a"])
            stt(s[1], g[1], hi, 22, dsem["g1b"])
            stt(bn[1], g[1], lo, 23, dsem["bn1a"])
            stt(bn[1], g[1], hi, 24, dsem["bn1b"])
            for i in range(4):  # gtt1 quarters are the stream tail
                stt_max(tm[1], gt[1], qs[i], 25 + i, 3, dsem[f"gt1q{i + 1}"])
            assert sv_n == 17

    return nc


def _numpy_reference(outputs, gt_shrink_labels, gt_threshold_labels):
    """Exact fallback for inputs outside the fast-path regime."""
    OHEM_RATIO, EPS = 3, 1e-7

    def sigmoid(x):
        return 1.0 / (1.0 + np.exp(-x))

    shrink, thresh, binary = outputs[:, 0], outputs[:, 1], outputs[:, 2]
    b = outputs.shape[0]
    flat_s = shrink.reshape(b, -1)
    flat_pos = (gt_shrink_labels > 0.5).reshape(b, -1)
    n = flat_s.shape[1]
    pos_num = flat_pos.sum(axis=1)
    neg_total = n - pos_num
    neg_num = np.minimum(pos_num * OHEM_RATIO, neg_total)
    neg_scores = np.where(flat_pos, -np.inf, flat_s)
    sorted_desc = -np.sort(-neg_scores, axis=1)
    idx = np.clip(neg_num - 1, 0, n - 1).astype(np.int64)
    thr = np.take_along_axis(sorted_desc, idx[:, None], axis=1)
    mask = (flat_s >= thr) | flat_pos
    valid = (pos_num > 0) & (neg_num > 0)
    mask = (mask & valid[:, None]).reshape(shrink.shape).astype(np.float32)

    def masked_bce(logits, target, m):
        p = np.clip(sigmoid(logits), EPS, 1.0 - EPS)
        t = (target > 0.5).astype(np.float32)
        per_px = -(t * np.log(p) + (1.0 - t) * np.log(1.0 - p))
        denom = m.sum()
        return float(per_px.flatten() @ m.flatten() / max(denom, 1.0)) if denom > 0 else 0.0

    loss_shrink = masked_bce(shrink, gt_shrink_labels, mask)
    loss_binary = masked_bce(binary, gt_shrink_labels, mask)
    m2 = ((gt_threshold_labels > 0) | (gt_shrink_labels > 0)).astype(np.float32)
    denom2 = m2.sum()
    l1 = np.abs(sigmoid(thresh) - gt_threshold_labels).flatten() @ m2.flatten()
    loss_thresh = float(l1 / max(denom2, 1.0)) if denom2 > 0 else 0.0
    loss_all = loss_shrink + ALPHA * loss_binary + BETA * loss_thresh
    return np.array([loss_all, loss_shrink, loss_binary, loss_thresh], np.float32)


def kernel(outputs, gt_shrink_labels, gt_threshold_labels, _trace=False):
    global _CACHED_NC
    outputs = np.ascontiguousarray(np.asarray(outputs, dtype=np.float32))
    gts = np.ascontiguousarray(np.asarray(gt_shrink_labels, dtype=np.float32))
    gtt = np.ascontiguousarray(np.asarray(gt_threshold_labels, dtype=np.float32))

    # ---- host-side regime checks (exactness guards for the fast path) ----
    pos_num = (gts > 0.5).reshape(B, -1).sum(axis=1)
    neg_total = N - pos_num
    neg_num = np.minimum(3 * pos_num, neg_total)
    valid = (pos_num > 0) & (neg_num > 0)
    needs_topk = valid & (3 * pos_num < neg_total)
    clip_active = max(
        float(np.abs(outputs[:, 0]).max()), float(np.abs(outputs[:, 2]).max())
    ) >= 16.0
    if needs_topk.any() or clip_active or not valid.all():
        return _numpy_reference(outputs, gts, gtt)

    if _CACHED_NC is None:
        _CACHED_NC = build_nc()
    nc = _CACHED_NC

    in_maps = []
    for c in range(NCORES):
        sl = slice(c * BPC, (c + 1) * BPC)
        in_maps.append({
            "outs": outputs[sl].reshape(BPC, 3, N),
            "gts": gts[sl].reshape(BPC, N),
            "gtt": gtt[sl].reshape(BPC, N),
        })
    res = run_bass_kernel_spmd(
        nc, in_maps, core_ids=list(range(NCORES)), trace=_trace
    )

    # ---- host combine: global sums from per-partition partials ----
    # sum(gtt) is a plain input reduction; the host computes it directly
    sum_g_all = float(gtt.astype(np.float64).sum())
    sp_s = sp_b = ts = tb = 0.0
    l1 = -sum_g_all
    for c in range(NCORES):
        po = res.results[c]["part"].astype(np.float64).sum(axis=0)
        sum_max = po[0] + po[25] + po[26] + po[27] + po[28]
        sum_u = po[9] + po[10] + po[11]
        l1 += 2.0 * sum_max - sum_u
        sp_s += po[1] + po[6]
        sp_b += po[2] + po[3] + po[7] + po[8]
        ts += po[13] + po[14] + po[15] + po[16] + po[21] + po[22]
        tb += po[17] + po[18] + po[19] + po[20] + po[23] + po[24]

    cnt = float(B * N)
    loss_shrink = (sp_s - ts) / cnt
    loss_binary = (sp_b - tb) / cnt

    # threshold-loss mask corrections for pixels where both labels <= 0
    zz = (gtt <= 0) & (gts <= 0)
    cnt2 = float(B * N - zz.sum())
    if zz.any():
        tmz = outputs[:, 1][zz]
        l1 -= float(np.abs(1.0 / (1.0 + np.exp(-tmz)) - gtt[zz]).sum())
    loss_thresh = l1 / max(cnt2, 1.0) if cnt2 > 0 else 0.0

    loss_all = loss_shrink + ALPHA * loss_binary + BETA * loss_thresh
    out = np.array([loss_all, loss_shrink, loss_binary, loss_thresh], np.float32)
    if _trace:
        return out, res
    return out
